# revision 1
# baseline (speedup 1.0000x reference)
"""Bass/Tile kernel builder for the two-stage attention block (v2).

Layout strategy: everything on-chip is kept transposed relative to the
reference ([feature, token] with feature on partitions) so that both
attention stages chain with zero on-chip transposes:

  QT/KT  = W.T @ x.T       : matmul(lhsT=W_chunk, rhs=xT_chunk)   -> [c, i]
  V      = x @ W           : matmul(lhsT=xT_chunk, rhs=Wv_chunk)  -> [j, c]
  S^T    = (q@k.T).T       : matmul(lhsT=KT_h, rhs=QT_h)          -> [j, i]
  P^T    = act(S^T * m^T)  : elementwise (orientation-agnostic)
  O^T    = (P@v).T         : matmul(lhsT=V_h, rhs=P^T_h)          -> [d, i]
  heads stacked on partitions -> O^T == out1.T, which directly feeds the
  next projection as lhsT chunks.

Softmax (stage 2) denominator: V2 is stored head-padded [j, h*128] with 64
ONES columns appended per head, so the apply matmul's PSUM rows 64:128 hold
sum_j exp(s) replicated across 64 partitions — the partition-broadcast of
the denominator is done by the matmul itself for free. Normalization is
then a [64, 512] reciprocal + multiply on the small apply output.

All matmul operands are float32r (~1.5e-4 rel err, full PE speed at N=512
vs 4x slowdown for fp32). f32r tiles are DMA'd straight from f32r-declared
DRAM inputs (raw fp32 bits; the PE rounds internally).

All DMAs ride HWDGE via nc.sync — SWDGE/gpsimd costs ~120us per op in this
environment and is avoided entirely.
"""

from contextlib import ExitStack

import concourse.bacc as bacc
import concourse.bass as bass
import concourse.tile as tile
from concourse import mybir
from concourse.vector_clock import ScopedClock

F32 = mybir.dt.float32
F32R = mybir.dt.float32r
AF = mybir.ActivationFunctionType
ALU = mybir.AluOpType

N, DIM, H, D = 1024, 512, 8, 64
SCALE = DIM**-0.5
KC = DIM // 128  # contraction chunks for projections
JC = N // 128  # key-side chunks (128 wide)
IC = N // 512  # query-side chunks (512 wide)
EXP_BIAS = -15.0
VP = 2 * D  # per-head width in padded V2: 64 data cols + 64 ones cols
_STOP_PHASE = 99  # internal: phase-bisect support (phase_bisect.py)


# ---------------------------------------------------------------------------
# Walrus in this container rejects instructions with >1 sync-wait.
# Split: hoist extra waits onto single-wait NoOps inserted just before.
def legalize_single_wait(nc):
    n_split = 0
    for fn in nc.m.functions:
        for blk in fn.blocks:
            insts = list(blk.instructions)
            out = []
            changed = False
            for inst in insts:
                si = inst.sync_info
                waits = list(si.on_wait) if (si is not None and si.on_wait) else []
                if len(waits) > 1:
                    changed = True
                    n_split += len(waits) - 1
                    for w in waits[:-1]:
                        nop = mybir.InstNoOp(
                            name=nc.get_next_instruction_name(),
                            sync_info=mybir.SyncInfo(on_wait=[w], on_update=[]),
                            bass_nofuse=True,
                            engine=inst.engine,
                        )
                        nc.register_instruction(nop)
                        out.append(nop)
                    si.on_wait = [waits[-1]]
                out.append(inst)
            if changed:
                blk.instructions = out
    return n_split


def _patched_drain_and_barrier(self, tick_clock, wait_clock):
    drain_inst = self.nc.sync.drain()
    wait_clock.add_sem_waits(
        drain_inst.ins, ScopedClock({None: tick_clock.global_clock})
    )
    si = drain_inst.ins.sync_info
    waits = list(si.on_wait or []) if si is not None else []
    if len(waits) > 1:
        si.on_wait = [waits[0]]
        for w in waits[1:]:
            extra = self.nc.sync.drain()
            esi = extra.ins.sync_info
            if esi is None:
                extra.ins.sync_info = mybir.SyncInfo(on_wait=[w], on_update=[])
            else:
                esi.on_wait = [w]

    self.nc.all_engine_barrier()
    assert self.sems is not None
    popped = self.nc._tile_sem_poison_stack.pop()
    assert popped is self._sem_poison
    self.nc.clear_and_free_semaphores(list(self.sems.allocated().values()))
    self.nc.all_engine_barrier()


def install_patches():
    tile.TileContext._drain_and_barrier = _patched_drain_and_barrier


# ---------------------------------------------------------------------------


def _qkv_proj_T(nc, pool_ps, w_sb, bias_sb, src_sb, dst, col0, pfx, pairs=(0, 1, 2, 3)):
    """dst[c, i] (c on partitions, 2 heads per tile) for cols [col0, col0+512)
    of the weight: dst = W[:, col0:col0+512].T @ src + b."""
    for t in pairs:  # c-chunks of 128 (head pairs)
        for ic in range(IC):
            ps = pool_ps.tile([128, 512], F32, tag="proj_ps", name=f"{pfx}_{t}_{ic}")
            for kc in range(KC):
                nc.tensor.matmul(
                    ps[:],
                    w_sb[kc][:, col0 + t * 128 : col0 + (t + 1) * 128],
                    src_sb[kc][:, ic * 512 : (ic + 1) * 512],
                    start=(kc == 0),
                    stop=(kc == KC - 1),
                )
            nc.vector.tensor_scalar_add(
                dst[t][:, ic * 512 : (ic + 1) * 512], ps[:], bias_sb[t][:]
            )


def build_body(ctx: ExitStack, tc: tile.TileContext, d, out_ap, taps=None):
    nc = tc.nc

    def tap(name, ap):
        if taps is not None and name in taps:
            if ap.dtype != F32:
                ap = ap.bitcast(F32)
            nc.sync.dma_start(taps[name][:], ap)

    const_pool = ctx.enter_context(tc.tile_pool(name="const", bufs=1))

    # --- persistent small constants (HWDGE loads, no gpsimd) --------------
    def load_bias_cols(name, src, off):
        """[128,1] per c-chunk bias tiles (c on partitions)."""
        tiles = []
        for t in range(4):
            b = const_pool.tile([128, 1], F32, name=f"{name}_{t}")
            nc.sync.dma_start(b[:], src[off + t * 128 : off + (t + 1) * 128])
            tiles.append(b)
        return tiles

    def load_full(name, src, shape, dtype):
        t = const_pool.tile(shape, dtype, name=name)
        nc.sync.dma_start(t[:], src[:, :])
        return t

    bq1 = load_bias_cols("bq1", d["bqkv1"], 0)
    bk1 = load_bias_cols("bk1", d["bqkv1"], DIM)
    bq2 = load_bias_cols("bq2", d["bqkv2"], 0)
    bk2 = load_bias_cols("bk2", d["bqkv2"], DIM)
    # host-prepared broadcast bias planes [128, DIM]
    bv1_b = load_full("bv1b", d["bv1b"], [128, DIM], F32)
    bv2_b = load_full("bv2b", d["bv2b"], [128, DIM], F32)
    bnn_b = load_full("bnnb", d["bnnb"], [128, DIM], F32)

    expb = const_pool.tile([128, 1], F32, name="expb")
    nc.vector.memset(expb[:], EXP_BIAS)

    # --- tensors that span stage boundaries -------------------------------
    o1_pool = ctx.enter_context(tc.tile_pool(name="o1", bufs=1))
    O1T = [o1_pool.tile([128, N], F32R, name=f"O1T_{t}") for t in range(4)]

    s1 = ctx.enter_context(ExitStack())  # stage-1 scope: closed after phase 2
    qk1_pool = s1.enter_context(tc.tile_pool(name="qk1", bufs=1))
    QT1 = [qk1_pool.tile([128, N], F32R, name=f"QT1_{t}") for t in range(4)]
    KT1 = [qk1_pool.tile([128, N], F32R, name=f"KT1_{t}") for t in range(4)]
    V1 = [qk1_pool.tile([128, DIM], F32R, name=f"V1_{j}") for j in range(JC)]

    mask_pool = s1.enter_context(tc.tile_pool(name="mask", bufs=1))
    maskT = [mask_pool.tile([128, N], F32, name=f"maskT_{j}") for j in range(JC)]

    # =====================================================================
    # Phase 1: stage-1 projections
    # =====================================================================
    with tc.tile_pool(name="xw1", bufs=1) as xw1_pool, \
         tc.tile_pool(name="ps1", bufs=4, space="PSUM") as ps1_pool:
        xT = [xw1_pool.tile([128, N], F32R, name=f"xT_{k}") for k in range(KC)]
        for k in range(KC):
            nc.scalar.dma_start(xT[k][:], d["xT"][k * 128 : (k + 1) * 128, :])
        W1 = [xw1_pool.tile([128, 3 * DIM], F32R, name=f"W1_{k}") for k in range(KC)]
        for blk in range(3):  # q, k, v column blocks — earliest-needed first
            for k in range(KC):
                nc.sync.dma_start(
                    W1[k][:, blk * DIM : (blk + 1) * DIM],
                    d["Wqkv1"][k * 128 : (k + 1) * 128, blk * DIM : (blk + 1) * DIM],
                )

        for j in range(JC):
            nc.scalar.dma_start(maskT[j][:], d["maskT"][j * 128 : (j + 1) * 128, :])

        # head-pair-0 Q/K first so pair-0 scores can start ASAP, then V
        # (pair-0 apply needs it), then the remaining pairs.
        _qkv_proj_T(nc, ps1_pool, W1, bq1, xT, QT1, 0, "q1", pairs=(0,))
        _qkv_proj_T(nc, ps1_pool, W1, bk1, xT, KT1, DIM, "k1", pairs=(0,))
        for j in range(JC):
            ps = ps1_pool.tile([128, 512], F32, tag="proj_ps", name=f"vps_{j}")
            for kc in range(KC):
                nc.tensor.matmul(
                    ps[:],
                    xT[kc][:, j * 128 : (j + 1) * 128],
                    W1[kc][:, 2 * DIM : 3 * DIM],
                    start=(kc == 0),
                    stop=(kc == KC - 1),
                )
            nc.vector.tensor_tensor(V1[j][:], ps[:], bv1_b[:], ALU.add)
        for t in range(1, 4):
            _qkv_proj_T(nc, ps1_pool, W1, bq1, xT, QT1, 0, "q1", pairs=(t,))
            _qkv_proj_T(nc, ps1_pool, W1, bk1, xT, KT1, DIM, "k1", pairs=(t,))

        tap("qt1_0", QT1[0][:])
        tap("v1_0", V1[0][:])

    if _STOP_PHASE <= 1:
        raise StopIteration

    # =====================================================================
    # Phase 2: stage-1 attention (sigmoid(S * mask) @ V), transposed
    # =====================================================================
    with tc.tile_pool(name="p1", bufs=18) as p_pool, \
         tc.tile_pool(name="ptmp", bufs=4) as ptmp_pool, \
         tc.tile_pool(name="sps1", bufs=2, space="PSUM") as score_ps, \
         tc.tile_pool(name="aps1", bufs=4, space="PSUM") as apply_ps:
        def emit_apply1(pt_, Pp, aps, j):
            for ic in range(IC):
                for h in (2 * pt_, 2 * pt_ + 1):
                    nc.tensor.matmul(
                        aps[h][ic][0:64, :],
                        V1[j][:, h * D : (h + 1) * D],
                        Pp[h][j][:, ic * 512 : (ic + 1) * 512],
                        start=(j == 0),
                        stop=(j == JC - 1),
                    )

        def evict_apply1(pt_, aps):
            for h in (2 * pt_, 2 * pt_ + 1):
                base = 64 * (h % 2)
                for ic in range(IC):
                    nc.scalar.copy(
                        O1T[pt_][base : base + 64, ic * 512 : (ic + 1) * 512],
                        aps[h][ic][0:64, :],
                    )

        prev = None  # (pair_idx, P, aps) — apply runs one pair behind scores
        for t in range(4):  # head pairs
            P = {}
            for h in (2 * t, 2 * t + 1):
                P[h] = [
                    p_pool.tile([128, N], F32R, tag="p", name=f"P1_{h}_{j}")
                    for j in range(JC)
                ]
            for j in range(JC):
                sps = {}
                for h in (2 * t, 2 * t + 1):
                    sps[h] = score_ps.tile(
                        [128, N], F32, tag="score_ps", name=f"sps_{h}_{j}"
                    )
                # alternate even/odd heads: disjoint PE row groups overlap
                for ic in range(IC):
                    for h in (2 * t, 2 * t + 1):
                        base = 64 * (h % 2)
                        nc.tensor.matmul(
                            sps[h][:, ic * 512 : (ic + 1) * 512],
                            KT1[t][base : base + 64, j * 128 : (j + 1) * 128],
                            QT1[t][base : base + 64, ic * 512 : (ic + 1) * 512],
                            start=True,
                            stop=True,
                        )
                for h in (2 * t, 2 * t + 1):
                    pt = ptmp_pool.tile([128, N], F32, tag="ptmp", name=f"pt_{h}_{j}")
                    nc.vector.tensor_tensor(pt[:], sps[h][:], maskT[j][:], ALU.mult)
                    nc.scalar.activation(P[h][j][:], pt[:], AF.Sigmoid)
                if prev is not None:
                    emit_apply1(prev[0], prev[1], prev[2], j)
            if prev is not None:
                evict_apply1(prev[0], prev[2])
            aps = {}
            for h in (2 * t, 2 * t + 1):
                aps[h] = [
                    apply_ps.tile([128, 512], F32, tag="apply_ps", name=f"aps1_{h}_{i}")
                    for i in range(IC)
                ]
            prev = (t, P, aps)
        for j in range(JC):
            emit_apply1(prev[0], prev[1], prev[2], j)
        evict_apply1(prev[0], prev[2])


    tap("o1t_0", O1T[0][:])

    if _STOP_PHASE <= 2:
        raise StopIteration
    s1.close()  # free QT1/KT1/V1/maskT

    # =====================================================================
    # Phase 3: stage-2 projections (from O1T)
    # =====================================================================
    qk2_pool = ctx.enter_context(tc.tile_pool(name="qk2", bufs=1))
    QT2 = [qk2_pool.tile([128, N], F32R, name=f"QT2_{t}") for t in range(4)]
    KT2 = [qk2_pool.tile([128, N], F32R, name=f"KT2_{t}") for t in range(4)]
    V2p = [qk2_pool.tile([128, H * VP], F32R, name=f"V2p_{j}") for j in range(JC)]

    # phase-4 pools open BEFORE ps2 so the score pool gets PSUM banks
    # disjoint from the projection pool — otherwise bank reuse serializes
    # the hoisted pair-0 scores behind all projections.
    s4 = ExitStack()
    o2_pool = ctx.enter_context(tc.tile_pool(name="o2", bufs=1))
    O2T = [o2_pool.tile([128, N], F32R, name=f"O2T_{t}") for t in range(4)]
    p2_pool = s4.enter_context(tc.tile_pool(name="p2", bufs=16))
    d_pool = s4.enter_context(tc.tile_pool(name="dscr", bufs=4))
    score2_ps = s4.enter_context(tc.tile_pool(name="sps2", bufs=2, space="PSUM"))

    def alloc_P2(t):
        return {
            h: [
                p2_pool.tile([128, N], F32R, tag="p2", name=f"P2_{h}_{j}")
                for j in range(JC)
            ]
            for h in (2 * t, 2 * t + 1)
        }

    def emit_scores2(t, P2d):
        for j in range(JC):
            sps = {}
            for h in (2 * t, 2 * t + 1):
                sps[h] = score2_ps.tile(
                    [128, N], F32, tag="score2_ps", name=f"s2ps_{h}_{j}"
                )
            for ic in range(IC):
                for h in (2 * t, 2 * t + 1):
                    base = 64 * (h % 2)
                    nc.tensor.matmul(
                        sps[h][:, ic * 512 : (ic + 1) * 512],
                        KT2[t][base : base + 64, j * 128 : (j + 1) * 128],
                        QT2[t][base : base + 64, ic * 512 : (ic + 1) * 512],
                        start=True,
                        stop=True,
                    )
            for h in (2 * t, 2 * t + 1):
                nc.scalar.activation(
                    P2d[h][j][:], sps[h][:], AF.Exp, bias=expb[:], scale=SCALE
                )

    with tc.tile_pool(name="w2", bufs=1) as w2_pool, \
         tc.tile_pool(name="ps2", bufs=4, space="PSUM") as ps2_pool:
        W2 = [w2_pool.tile([128, 3 * DIM], F32R, name=f"W2_{k}") for k in range(KC)]
        for blk in range(3):
            for k in range(KC):
                nc.sync.dma_start(
                    W2[k][:, blk * DIM : (blk + 1) * DIM],
                    d["Wqkv2"][k * 128 : (k + 1) * 128, blk * DIM : (blk + 1) * DIM],
                )

        _qkv_proj_T(nc, ps2_pool, W2, bq2, O1T, QT2, 0, "q2", pairs=(0,))
        _qkv_proj_T(nc, ps2_pool, W2, bk2, O1T, KT2, DIM, "k2", pairs=(0,))
        # hoisted: pair-0 stage-2 scores+exp overlap the remaining projections
        P2d0 = alloc_P2(0)
        emit_scores2(0, P2d0)
        for t in range(1, 4):
            _qkv_proj_T(nc, ps2_pool, W2, bq2, O1T, QT2, 0, "q2", pairs=(t,))
            _qkv_proj_T(nc, ps2_pool, W2, bk2, O1T, KT2, DIM, "k2", pairs=(t,))
        for j in range(JC):
            ps = ps2_pool.tile([128, 512], F32, tag="proj_ps", name=f"v2ps_{j}")
            for kc in range(KC):
                nc.tensor.matmul(
                    ps[:],
                    O1T[kc][:, j * 128 : (j + 1) * 128],
                    W2[kc][:, 2 * DIM : 3 * DIM],
                    start=(kc == 0),
                    stop=(kc == KC - 1),
                )
            # scatter per-head into the padded layout [j, h*128 + d]
            nc.vector.tensor_tensor(
                V2p[j][:, :].rearrange("p (h e) -> p h e", e=VP)[:, :, :D],
                ps[:].rearrange("p (h dd) -> p h dd", dd=D),
                bv2_b[:].rearrange("p (h dd) -> p h dd", dd=D),
                ALU.add,
            )
            # 64 ones columns per head (drives matmul-replicated denominators)
            nc.sync.dma_start(
                V2p[j][:, :].rearrange("p (h e) -> p h e", e=VP)[:, :, D:VP],
                d["onesb"][:, :].rearrange("p (h dd) -> p h dd", dd=D),
            )
        tap("qt2_0", QT2[0][:])
        tap("v2p_0", V2p[0][:])

    if _STOP_PHASE <= 3:
        raise StopIteration

    # =====================================================================
    # Phase 4: stage-2 attention (softmax via exp + replicated denominators)
    # pair-0 scores were hoisted into phase 3; apply2 psum pool opens only
    # now (reusing the projection pool's banks).
    # =====================================================================
    apply2_ps = s4.enter_context(tc.tile_pool(name="aps2", bufs=4, space="PSUM"))

    def emit_apply2(pt_, Pp, aps, j):
        # apply with 64 ones cols: PSUM rows 0:64 = unnormalized out,
        # rows 64:128 = softmax denominator replicated 64x
        for h in (2 * pt_, 2 * pt_ + 1):
            for ic in range(IC):
                nc.tensor.matmul(
                    aps[h][ic][:, :],
                    V2p[j][:, h * VP : (h + 1) * VP],
                    Pp[h][j][:, ic * 512 : (ic + 1) * 512],
                    start=(j == 0),
                    stop=(j == JC - 1),
                )

    def evict_apply2(pt_, aps):
        for h in (2 * pt_, 2 * pt_ + 1):
            base = 64 * (h % 2)
            for ic in range(IC):
                db = d_pool.tile([64, 512], F32, tag="db", name=f"db_{h}_{ic}")
                nc.vector.reciprocal(db[:], aps[h][ic][64:128, :])
                if h == 0 and ic == 0:
                    tap("db_00", db[:])
                nc.vector.tensor_tensor(
                    O2T[pt_][base : base + 64, ic * 512 : (ic + 1) * 512],
                    aps[h][ic][0:64, :],
                    db[:],
                    ALU.mult,
                )

    def alloc_aps2(t):
        return {
            h: [
                apply2_ps.tile(
                    [128, 512], F32, tag="apply2_ps", name=f"aps2_{h}_{i}"
                )
                for i in range(IC)
            ]
            for h in (2 * t, 2 * t + 1)
        }

    prev = (0, P2d0, alloc_aps2(0))  # pair-0 scores already emitted in phase 3
    for t in range(1, 4):
        P2d = alloc_P2(t)
        for j in range(JC):
            # local closure: emit scores for (t, j) then apply for prev pair
            sps = {}
            for h in (2 * t, 2 * t + 1):
                sps[h] = score2_ps.tile(
                    [128, N], F32, tag="score2_ps", name=f"s2ps_{h}_{j}"
                )
            for ic in range(IC):
                for h in (2 * t, 2 * t + 1):
                    base = 64 * (h % 2)
                    nc.tensor.matmul(
                        sps[h][:, ic * 512 : (ic + 1) * 512],
                        KT2[t][base : base + 64, j * 128 : (j + 1) * 128],
                        QT2[t][base : base + 64, ic * 512 : (ic + 1) * 512],
                        start=True,
                        stop=True,
                    )
            for h in (2 * t, 2 * t + 1):
                nc.scalar.activation(
                    P2d[h][j][:], sps[h][:], AF.Exp, bias=expb[:], scale=SCALE
                )
            emit_apply2(prev[0], prev[1], prev[2], j)
        evict_apply2(prev[0], prev[2])
        prev = (t, P2d, alloc_aps2(t))
    for j in range(JC):
        emit_apply2(prev[0], prev[1], prev[2], j)
    evict_apply2(prev[0], prev[2])
    s4.close()

    tap("o2t_0", O2T[0][:])

    if _STOP_PHASE <= 4:
        raise StopIteration

    # =====================================================================
    # Phase 5: output projection
    # =====================================================================
    with tc.tile_pool(name="wnn", bufs=1) as wnn_pool, \
         tc.tile_pool(name="outst", bufs=6) as out_pool, \
         tc.tile_pool(name="ps5", bufs=4, space="PSUM") as ps5_pool:
        Wnn = [wnn_pool.tile([128, DIM], F32R, name=f"Wnn_{k}") for k in range(KC)]
        for k in range(KC):
            nc.sync.dma_start(Wnn[k][:], d["Wnn1"][k * 128 : (k + 1) * 128, :])
        for i8 in range(JC):  # 8 chunks of 128 output rows
            ps = ps5_pool.tile([128, 512], F32, tag="out_ps", name=f"ops_{i8}")
            for kc in range(KC):
                nc.tensor.matmul(
                    ps[:],
                    O2T[kc][:, i8 * 128 : (i8 + 1) * 128],
                    Wnn[kc][:],
                    start=(kc == 0),
                    stop=(kc == KC - 1),
                )
            ob = out_pool.tile([128, DIM], F32, tag="ob", name=f"ob_{i8}")
            nc.vector.tensor_tensor(ob[:], ps[:], bnn_b[:], ALU.add)
            nc.sync.dma_start(out_ap[i8 * 128 : (i8 + 1) * 128, :], ob[:])


def build(n_repeat: int = 1, debug_taps: bool = False):
    install_patches()
    nc = bacc.Bacc("TRN2", target_bir_lowering=False, debug=False)
    d = {}

    def din(name, shape, dtype=F32):
        d[name] = nc.dram_tensor(name, shape, dtype, kind="ExternalInput").ap()

    din("xT", [DIM, N], F32R)
    din("maskT", [N, N])
    din("Wqkv1", [DIM, 3 * DIM], F32R)
    din("bqkv1", [3 * DIM])
    din("Wqkv2", [DIM, 3 * DIM], F32R)
    din("bqkv2", [3 * DIM])
    din("Wnn1", [DIM, DIM], F32R)
    din("bnn1", [DIM])
    din("bv1b", [128, DIM])
    din("bv2b", [128, DIM])
    din("bnnb", [128, DIM])
    din("onesb", [128, DIM], F32R)
    out_ap = nc.dram_tensor("out", [N, DIM], F32, kind="ExternalOutput").ap()

    taps = None
    if debug_taps:
        shapes = {"qt1_0": [128, N], "v1_0": [128, DIM], "o1t_0": [128, N],
                  "qt2_0": [128, N], "v2p_0": [128, H * VP],
                  "db_00": [64, 512], "o2t_0": [128, N]}
        taps = {k: nc.dram_tensor(f"tap_{k}", v, F32, kind="ExternalOutput").ap()
                for k, v in shapes.items()}

    with tile.TileContext(nc) as tc:
        for _ in range(n_repeat):
            with ExitStack() as ctx:
                try:
                    build_body(ctx, tc, d, out_ap, taps=taps)
                except StopIteration:
                    pass

    nc.compile()
    n = legalize_single_wait(nc)
    return nc, n


# ===========================================================================
# Host-side entry point: full inputs in, full output out.
# Sharding: pure data-parallel — B=8 batch elements, one per NeuronCore.
# ===========================================================================
import numpy as np

_CACHED = {}


def _get_program():
    if "nc" not in _CACHED:
        _CACHED["nc"] = build(n_repeat=1)[0]
    return _CACHED["nc"]


def _make_common(mask, Wqkv1, bqkv1, Wqkv2, bqkv2, Wnn1, bnn1):
    f32 = lambda a: np.ascontiguousarray(np.asarray(a, dtype=np.float32))
    bqkv1, bqkv2, bnn1 = f32(bqkv1), f32(bqkv2), f32(bnn1)
    return {
        "maskT": f32(np.asarray(mask)[0, 0].T),
        "Wqkv1": f32(Wqkv1),
        "bqkv1": bqkv1,
        "Wqkv2": f32(Wqkv2),
        "bqkv2": bqkv2,
        "Wnn1": f32(Wnn1),
        "bnn1": bnn1,
        "bv1b": f32(np.broadcast_to(bqkv1[2 * DIM :], (128, DIM))),
        "bv2b": f32(np.broadcast_to(bqkv2[2 * DIM :], (128, DIM))),
        "bnnb": f32(np.broadcast_to(bnn1, (128, DIM))),
        "onesb": np.ones((128, DIM), dtype=np.float32),
    }


def kernel(x, mask, Wqkv1, bqkv1, Wqkv2, bqkv2, Wnn1, bnn1):
    from concourse.bass_utils import run_bass_kernel_spmd

    x = np.asarray(x, dtype=np.float32)
    common = _make_common(mask, Wqkv1, bqkv1, Wqkv2, bqkv2, Wnn1, bnn1)
    in_maps = [
        {"xT": np.ascontiguousarray(x[c].T), **common} for c in range(x.shape[0])
    ]
    nc = _get_program()
    res = run_bass_kernel_spmd(nc, in_maps, core_ids=list(range(8)))
    return np.stack([res.results[c]["out"] for c in range(8)]).astype(np.float32)



# revision 2
# speedup vs baseline: 8.1835x; 8.1835x over previous
"""Bass/Tile kernel builder for the two-stage attention block (v2).

Layout strategy: everything on-chip is kept transposed relative to the
reference ([feature, token] with feature on partitions) so that both
attention stages chain with zero on-chip transposes:

  QT/KT  = W.T @ x.T       : matmul(lhsT=W_chunk, rhs=xT_chunk)   -> [c, i]
  V      = x @ W           : matmul(lhsT=xT_chunk, rhs=Wv_chunk)  -> [j, c]
  S^T    = (q@k.T).T       : matmul(lhsT=KT_h, rhs=QT_h)          -> [j, i]
  P^T    = act(S^T * m^T)  : elementwise (orientation-agnostic)
  O^T    = (P@v).T         : matmul(lhsT=V_h, rhs=P^T_h)          -> [d, i]
  heads stacked on partitions -> O^T == out1.T, which directly feeds the
  next projection as lhsT chunks.

Softmax (stage 2) denominator: V2 is stored head-padded [j, h*128] with 64
ONES columns appended per head, so the apply matmul's PSUM rows 64:128 hold
sum_j exp(s) replicated across 64 partitions — the partition-broadcast of
the denominator is done by the matmul itself for free. Normalization is
then a [64, 512] reciprocal + multiply on the small apply output.

All matmul operands are float32r (~1.5e-4 rel err, full PE speed at N=512
vs 4x slowdown for fp32). f32r tiles are DMA'd straight from f32r-declared
DRAM inputs (raw fp32 bits; the PE rounds internally).

All DMAs ride HWDGE via nc.sync — SWDGE/gpsimd costs ~120us per op in this
environment and is avoided entirely.
"""

from contextlib import ExitStack

import concourse.bacc as bacc
import concourse.bass as bass
import concourse.tile as tile
from concourse import mybir
from concourse.vector_clock import ScopedClock

F32 = mybir.dt.float32
F32R = mybir.dt.float32r
AF = mybir.ActivationFunctionType
ALU = mybir.AluOpType

N, DIM, H, D = 1024, 512, 8, 64
SCALE = DIM**-0.5
KC = DIM // 128  # contraction chunks for projections
JC = N // 128  # key-side chunks (128 wide)
IC = N // 512  # query-side chunks (512 wide)
EXP_BIAS = -15.0
VP = 2 * D  # per-head width in padded V2: 64 data cols + 64 ones cols
_STOP_PHASE = 99  # internal: phase-bisect support (phase_bisect.py)


# ---------------------------------------------------------------------------
# Walrus in this container rejects instructions with >1 sync-wait.
# Split: hoist extra waits onto single-wait NoOps inserted just before.
def legalize_single_wait(nc):
    n_split = 0
    for fn in nc.m.functions:
        for blk in fn.blocks:
            insts = list(blk.instructions)
            out = []
            changed = False
            for inst in insts:
                si = inst.sync_info
                waits = list(si.on_wait) if (si is not None and si.on_wait) else []
                if len(waits) > 1:
                    changed = True
                    n_split += len(waits) - 1
                    for w in waits[:-1]:
                        nop = mybir.InstNoOp(
                            name=nc.get_next_instruction_name(),
                            sync_info=mybir.SyncInfo(on_wait=[w], on_update=[]),
                            bass_nofuse=True,
                            engine=inst.engine,
                        )
                        nc.register_instruction(nop)
                        out.append(nop)
                    si.on_wait = [waits[-1]]
                out.append(inst)
            if changed:
                blk.instructions = out
    return n_split


def _patched_drain_and_barrier(self, tick_clock, wait_clock):
    drain_inst = self.nc.sync.drain()
    wait_clock.add_sem_waits(
        drain_inst.ins, ScopedClock({None: tick_clock.global_clock})
    )
    si = drain_inst.ins.sync_info
    waits = list(si.on_wait or []) if si is not None else []
    if len(waits) > 1:
        si.on_wait = [waits[0]]
        for w in waits[1:]:
            extra = self.nc.sync.drain()
            esi = extra.ins.sync_info
            if esi is None:
                extra.ins.sync_info = mybir.SyncInfo(on_wait=[w], on_update=[])
            else:
                esi.on_wait = [w]

    self.nc.all_engine_barrier()
    assert self.sems is not None
    popped = self.nc._tile_sem_poison_stack.pop()
    assert popped is self._sem_poison
    self.nc.clear_and_free_semaphores(list(self.sems.allocated().values()))
    self.nc.all_engine_barrier()


def install_patches():
    tile.TileContext._drain_and_barrier = _patched_drain_and_barrier


# ---------------------------------------------------------------------------


def _qkv_proj_T(nc, pool_ps, w_sb, bias_sb, src_sb, dst, col0, pfx, pairs=(0, 1, 2, 3)):
    """dst[c, i] (c on partitions, 2 heads per tile) for cols [col0, col0+512)
    of the weight: dst = W[:, col0:col0+512].T @ src + b."""
    for t in pairs:  # c-chunks of 128 (head pairs)
        for ic in range(IC):
            ps = pool_ps.tile([128, 512], F32, tag="proj_ps", name=f"{pfx}_{t}_{ic}")
            for kc in range(KC):
                nc.tensor.matmul(
                    ps[:],
                    w_sb[kc][:, col0 + t * 128 : col0 + (t + 1) * 128],
                    src_sb[kc][:, ic * 512 : (ic + 1) * 512],
                    start=(kc == 0),
                    stop=(kc == KC - 1),
                )
            nc.vector.tensor_scalar_add(
                dst[t][:, ic * 512 : (ic + 1) * 512], ps[:], bias_sb[t][:]
            )


def build_body(ctx: ExitStack, tc: tile.TileContext, d, out_ap, taps=None):
    nc = tc.nc

    def tap(name, ap):
        if taps is not None and name in taps:
            if ap.dtype != F32:
                ap = ap.bitcast(F32)
            nc.sync.dma_start(taps[name][:], ap)

    const_pool = ctx.enter_context(tc.tile_pool(name="const", bufs=1))

    # --- persistent small constants (HWDGE loads, no gpsimd) --------------
    def load_bias_cols(name, src, off):
        """[128,1] per c-chunk bias tiles (c on partitions)."""
        tiles = []
        for t in range(4):
            b = const_pool.tile([128, 1], F32, name=f"{name}_{t}")
            nc.sync.dma_start(b[:], src[off + t * 128 : off + (t + 1) * 128])
            tiles.append(b)
        return tiles

    def load_full(name, src, shape, dtype):
        t = const_pool.tile(shape, dtype, name=name)
        nc.sync.dma_start(t[:], src[:, :])
        return t

    bq1 = load_bias_cols("bq1", d["bqkv1"], 0)
    bk1 = load_bias_cols("bk1", d["bqkv1"], DIM)
    bq2 = load_bias_cols("bq2", d["bqkv2"], 0)
    bk2 = load_bias_cols("bk2", d["bqkv2"], DIM)
    # host-prepared broadcast bias planes [128, DIM]
    bv1_b = load_full("bv1b", d["bv1b"], [128, DIM], F32)
    bv2_b = load_full("bv2b", d["bv2b"], [128, DIM], F32)
    bnn_b = load_full("bnnb", d["bnnb"], [128, DIM], F32)

    expb = const_pool.tile([128, 1], F32, name="expb")
    nc.vector.memset(expb[:], EXP_BIAS)

    # --- tensors that span stage boundaries -------------------------------
    o1_pool = ctx.enter_context(tc.tile_pool(name="o1", bufs=1))
    O1T = [o1_pool.tile([128, N], F32R, name=f"O1T_{t}") for t in range(4)]

    s1 = ctx.enter_context(ExitStack())  # stage-1 scope: closed after phase 2
    qk1_pool = s1.enter_context(tc.tile_pool(name="qk1", bufs=1))
    QT1 = [qk1_pool.tile([128, N], F32R, name=f"QT1_{t}") for t in range(4)]
    KT1 = [qk1_pool.tile([128, N], F32R, name=f"KT1_{t}") for t in range(4)]
    V1 = [qk1_pool.tile([128, DIM], F32R, name=f"V1_{j}") for j in range(JC)]

    mask_pool = s1.enter_context(tc.tile_pool(name="mask", bufs=1))
    maskT = [mask_pool.tile([128, N], F32, name=f"maskT_{j}") for j in range(JC)]

    # =====================================================================
    # Phase 1: stage-1 projections
    # =====================================================================
    with tc.tile_pool(name="xw1", bufs=1) as xw1_pool, \
         tc.tile_pool(name="ps1", bufs=4, space="PSUM") as ps1_pool:
        xT = [xw1_pool.tile([128, N], F32R, name=f"xT_{k}") for k in range(KC)]
        for k in range(KC):
            nc.scalar.dma_start(xT[k][:], d["xT"][k * 128 : (k + 1) * 128, :])
        W1 = [xw1_pool.tile([128, 3 * DIM], F32R, name=f"W1_{k}") for k in range(KC)]
        for blk in range(3):  # q, k, v column blocks — earliest-needed first
            for k in range(KC):
                nc.sync.dma_start(
                    W1[k][:, blk * DIM : (blk + 1) * DIM],
                    d["Wqkv1"][k * 128 : (k + 1) * 128, blk * DIM : (blk + 1) * DIM],
                )

        for j in range(JC):
            nc.scalar.dma_start(maskT[j][:], d["maskT"][j * 128 : (j + 1) * 128, :])

        # head-pair-0 Q/K first so pair-0 scores can start ASAP, then V
        # (pair-0 apply needs it), then the remaining pairs.
        _qkv_proj_T(nc, ps1_pool, W1, bq1, xT, QT1, 0, "q1", pairs=(0,))
        _qkv_proj_T(nc, ps1_pool, W1, bk1, xT, KT1, DIM, "k1", pairs=(0,))
        for j in range(JC):
            ps = ps1_pool.tile([128, 512], F32, tag="proj_ps", name=f"vps_{j}")
            for kc in range(KC):
                nc.tensor.matmul(
                    ps[:],
                    xT[kc][:, j * 128 : (j + 1) * 128],
                    W1[kc][:, 2 * DIM : 3 * DIM],
                    start=(kc == 0),
                    stop=(kc == KC - 1),
                )
            nc.vector.tensor_tensor(V1[j][:], ps[:], bv1_b[:], ALU.add)
        for t in range(1, 4):
            _qkv_proj_T(nc, ps1_pool, W1, bq1, xT, QT1, 0, "q1", pairs=(t,))
            _qkv_proj_T(nc, ps1_pool, W1, bk1, xT, KT1, DIM, "k1", pairs=(t,))

        tap("qt1_0", QT1[0][:])
        tap("v1_0", V1[0][:])

    if _STOP_PHASE <= 1:
        raise StopIteration

    # =====================================================================
    # Phase 2: stage-1 attention (sigmoid(S * mask) @ V), transposed
    # =====================================================================
    with tc.tile_pool(name="p1", bufs=18) as p_pool, \
         tc.tile_pool(name="ptmp", bufs=4) as ptmp_pool, \
         tc.tile_pool(name="sps1", bufs=2, space="PSUM") as score_ps, \
         tc.tile_pool(name="aps1", bufs=4, space="PSUM") as apply_ps:
        def emit_apply1(pt_, Pp, aps, j):
            for ic in range(IC):
                for h in (2 * pt_, 2 * pt_ + 1):
                    nc.tensor.matmul(
                        aps[h][ic][0:64, :],
                        V1[j][:, h * D : (h + 1) * D],
                        Pp[h][j][:, ic * 512 : (ic + 1) * 512],
                        start=(j == 0),
                        stop=(j == JC - 1),
                    )

        def evict_apply1(pt_, aps):
            for h in (2 * pt_, 2 * pt_ + 1):
                base = 64 * (h % 2)
                for ic in range(IC):
                    nc.scalar.copy(
                        O1T[pt_][base : base + 64, ic * 512 : (ic + 1) * 512],
                        aps[h][ic][0:64, :],
                    )

        prev = None  # (pair_idx, P, aps) — apply runs one pair behind scores
        for t in range(4):  # head pairs
            P = {}
            for h in (2 * t, 2 * t + 1):
                P[h] = [
                    p_pool.tile([128, N], F32R, tag="p", name=f"P1_{h}_{j}")
                    for j in range(JC)
                ]
            for j in range(JC):
                sps = {}
                for h in (2 * t, 2 * t + 1):
                    sps[h] = score_ps.tile(
                        [128, N], F32, tag="score_ps", name=f"sps_{h}_{j}"
                    )
                # alternate even/odd heads: disjoint PE row groups overlap
                for ic in range(IC):
                    for h in (2 * t, 2 * t + 1):
                        base = 64 * (h % 2)
                        nc.tensor.matmul(
                            sps[h][:, ic * 512 : (ic + 1) * 512],
                            KT1[t][base : base + 64, j * 128 : (j + 1) * 128],
                            QT1[t][base : base + 64, ic * 512 : (ic + 1) * 512],
                            start=True,
                            stop=True,
                        )
                for h in (2 * t, 2 * t + 1):
                    pt = ptmp_pool.tile([128, N], F32, tag="ptmp", name=f"pt_{h}_{j}")
                    nc.vector.tensor_tensor(pt[:], sps[h][:], maskT[j][:], ALU.mult)
                    nc.scalar.activation(P[h][j][:], pt[:], AF.Sigmoid)
                if prev is not None:
                    emit_apply1(prev[0], prev[1], prev[2], j)
            if prev is not None:
                evict_apply1(prev[0], prev[2])
            aps = {}
            for h in (2 * t, 2 * t + 1):
                aps[h] = [
                    apply_ps.tile([128, 512], F32, tag="apply_ps", name=f"aps1_{h}_{i}")
                    for i in range(IC)
                ]
            prev = (t, P, aps)
        for j in range(JC):
            emit_apply1(prev[0], prev[1], prev[2], j)
        evict_apply1(prev[0], prev[2])


    tap("o1t_0", O1T[0][:])

    if _STOP_PHASE <= 2:
        raise StopIteration
    s1.close()  # free QT1/KT1/V1/maskT

    # =====================================================================
    # Phase 3: stage-2 projections (from O1T)
    # =====================================================================
    qk2_pool = ctx.enter_context(tc.tile_pool(name="qk2", bufs=1))
    QT2 = [qk2_pool.tile([128, N], F32R, name=f"QT2_{t}") for t in range(4)]
    KT2 = [qk2_pool.tile([128, N], F32R, name=f"KT2_{t}") for t in range(4)]
    V2p = [qk2_pool.tile([128, H * VP], F32R, name=f"V2p_{j}") for j in range(JC)]

    # phase-4 pools open BEFORE ps2 so the score pool gets PSUM banks
    # disjoint from the projection pool — otherwise bank reuse serializes
    # the hoisted pair-0 scores behind all projections.
    s4 = ExitStack()
    o2_pool = ctx.enter_context(tc.tile_pool(name="o2", bufs=1))
    O2T = [o2_pool.tile([128, N], F32R, name=f"O2T_{t}") for t in range(4)]
    p2_pool = s4.enter_context(tc.tile_pool(name="p2", bufs=16))
    d_pool = s4.enter_context(tc.tile_pool(name="dscr", bufs=4))
    score2_ps = s4.enter_context(tc.tile_pool(name="sps2", bufs=2, space="PSUM"))

    def alloc_P2(t):
        return {
            h: [
                p2_pool.tile([128, N], F32R, tag="p2", name=f"P2_{h}_{j}")
                for j in range(JC)
            ]
            for h in (2 * t, 2 * t + 1)
        }

    def emit_scores2(t, P2d):
        for j in range(JC):
            sps = {}
            for h in (2 * t, 2 * t + 1):
                sps[h] = score2_ps.tile(
                    [128, N], F32, tag="score2_ps", name=f"s2ps_{h}_{j}"
                )
            for ic in range(IC):
                for h in (2 * t, 2 * t + 1):
                    base = 64 * (h % 2)
                    nc.tensor.matmul(
                        sps[h][:, ic * 512 : (ic + 1) * 512],
                        KT2[t][base : base + 64, j * 128 : (j + 1) * 128],
                        QT2[t][base : base + 64, ic * 512 : (ic + 1) * 512],
                        start=True,
                        stop=True,
                    )
            for h in (2 * t, 2 * t + 1):
                nc.scalar.activation(
                    P2d[h][j][:], sps[h][:], AF.Exp, bias=expb[:], scale=SCALE
                )

    with tc.tile_pool(name="w2", bufs=1) as w2_pool, \
         tc.tile_pool(name="ps2", bufs=4, space="PSUM") as ps2_pool:
        W2 = [w2_pool.tile([128, 3 * DIM], F32R, name=f"W2_{k}") for k in range(KC)]
        for blk in range(3):
            for k in range(KC):
                nc.sync.dma_start(
                    W2[k][:, blk * DIM : (blk + 1) * DIM],
                    d["Wqkv2"][k * 128 : (k + 1) * 128, blk * DIM : (blk + 1) * DIM],
                )

        _qkv_proj_T(nc, ps2_pool, W2, bq2, O1T, QT2, 0, "q2", pairs=(0,))
        _qkv_proj_T(nc, ps2_pool, W2, bk2, O1T, KT2, DIM, "k2", pairs=(0,))
        # hoisted: pair-0 stage-2 scores+exp overlap the remaining projections
        P2d0 = alloc_P2(0)
        emit_scores2(0, P2d0)
        for t in range(1, 4):
            _qkv_proj_T(nc, ps2_pool, W2, bq2, O1T, QT2, 0, "q2", pairs=(t,))
            _qkv_proj_T(nc, ps2_pool, W2, bk2, O1T, KT2, DIM, "k2", pairs=(t,))
        for j in range(JC):
            ps = ps2_pool.tile([128, 512], F32, tag="proj_ps", name=f"v2ps_{j}")
            for kc in range(KC):
                nc.tensor.matmul(
                    ps[:],
                    O1T[kc][:, j * 128 : (j + 1) * 128],
                    W2[kc][:, 2 * DIM : 3 * DIM],
                    start=(kc == 0),
                    stop=(kc == KC - 1),
                )
            # scatter per-head into the padded layout [j, h*128 + d]
            nc.vector.tensor_tensor(
                V2p[j][:, :].rearrange("p (h e) -> p h e", e=VP)[:, :, :D],
                ps[:].rearrange("p (h dd) -> p h dd", dd=D),
                bv2_b[:].rearrange("p (h dd) -> p h dd", dd=D),
                ALU.add,
            )
            # 64 ones columns per head (drives matmul-replicated denominators)
            nc.sync.dma_start(
                V2p[j][:, :].rearrange("p (h e) -> p h e", e=VP)[:, :, D:VP],
                d["onesb"][:, :].rearrange("p (h dd) -> p h dd", dd=D),
            )
        tap("qt2_0", QT2[0][:])
        tap("v2p_0", V2p[0][:])

    if _STOP_PHASE <= 3:
        raise StopIteration

    # =====================================================================
    # Phase 4: stage-2 attention (softmax via exp + replicated denominators)
    # pair-0 scores were hoisted into phase 3; apply2 psum pool opens only
    # now (reusing the projection pool's banks).
    # =====================================================================
    apply2_ps = s4.enter_context(tc.tile_pool(name="aps2", bufs=4, space="PSUM"))

    def emit_apply2(pt_, Pp, aps, j):
        # apply with 64 ones cols: PSUM rows 0:64 = unnormalized out,
        # rows 64:128 = softmax denominator replicated 64x
        for h in (2 * pt_, 2 * pt_ + 1):
            for ic in range(IC):
                nc.tensor.matmul(
                    aps[h][ic][:, :],
                    V2p[j][:, h * VP : (h + 1) * VP],
                    Pp[h][j][:, ic * 512 : (ic + 1) * 512],
                    start=(j == 0),
                    stop=(j == JC - 1),
                )

    def evict_apply2(pt_, aps):
        for h in (2 * pt_, 2 * pt_ + 1):
            base = 64 * (h % 2)
            for ic in range(IC):
                db = d_pool.tile([64, 512], F32, tag="db", name=f"db_{h}_{ic}")
                nc.vector.reciprocal(db[:], aps[h][ic][64:128, :])
                if h == 0 and ic == 0:
                    tap("db_00", db[:])
                nc.vector.tensor_tensor(
                    O2T[pt_][base : base + 64, ic * 512 : (ic + 1) * 512],
                    aps[h][ic][0:64, :],
                    db[:],
                    ALU.mult,
                )

    def alloc_aps2(t):
        return {
            h: [
                apply2_ps.tile(
                    [128, 512], F32, tag="apply2_ps", name=f"aps2_{h}_{i}"
                )
                for i in range(IC)
            ]
            for h in (2 * t, 2 * t + 1)
        }

    prev = (0, P2d0, alloc_aps2(0))  # pair-0 scores already emitted in phase 3
    for t in range(1, 4):
        P2d = alloc_P2(t)
        for j in range(JC):
            # local closure: emit scores for (t, j) then apply for prev pair
            sps = {}
            for h in (2 * t, 2 * t + 1):
                sps[h] = score2_ps.tile(
                    [128, N], F32, tag="score2_ps", name=f"s2ps_{h}_{j}"
                )
            for ic in range(IC):
                for h in (2 * t, 2 * t + 1):
                    base = 64 * (h % 2)
                    nc.tensor.matmul(
                        sps[h][:, ic * 512 : (ic + 1) * 512],
                        KT2[t][base : base + 64, j * 128 : (j + 1) * 128],
                        QT2[t][base : base + 64, ic * 512 : (ic + 1) * 512],
                        start=True,
                        stop=True,
                    )
            for h in (2 * t, 2 * t + 1):
                nc.scalar.activation(
                    P2d[h][j][:], sps[h][:], AF.Exp, bias=expb[:], scale=SCALE
                )
            emit_apply2(prev[0], prev[1], prev[2], j)
        evict_apply2(prev[0], prev[2])
        prev = (t, P2d, alloc_aps2(t))
    for j in range(JC):
        emit_apply2(prev[0], prev[1], prev[2], j)
    evict_apply2(prev[0], prev[2])
    s4.close()

    tap("o2t_0", O2T[0][:])

    if _STOP_PHASE <= 4:
        raise StopIteration

    # =====================================================================
    # Phase 5: output projection
    # =====================================================================
    with tc.tile_pool(name="wnn", bufs=1) as wnn_pool, \
         tc.tile_pool(name="outst", bufs=6) as out_pool, \
         tc.tile_pool(name="ps5", bufs=4, space="PSUM") as ps5_pool:
        Wnn = [wnn_pool.tile([128, DIM], F32R, name=f"Wnn_{k}") for k in range(KC)]
        for k in range(KC):
            nc.sync.dma_start(Wnn[k][:], d["Wnn1"][k * 128 : (k + 1) * 128, :])
        for i8 in range(JC):  # 8 chunks of 128 output rows
            ps = ps5_pool.tile([128, 512], F32, tag="out_ps", name=f"ops_{i8}")
            for kc in range(KC):
                nc.tensor.matmul(
                    ps[:],
                    O2T[kc][:, i8 * 128 : (i8 + 1) * 128],
                    Wnn[kc][:],
                    start=(kc == 0),
                    stop=(kc == KC - 1),
                )
            ob = out_pool.tile([128, DIM], F32, tag="ob", name=f"ob_{i8}")
            nc.vector.tensor_tensor(ob[:], ps[:], bnn_b[:], ALU.add)
            nc.sync.dma_start(out_ap[i8 * 128 : (i8 + 1) * 128, :], ob[:])


def build(n_repeat: int = 1, debug_taps: bool = False):
    install_patches()
    nc = bacc.Bacc("TRN2", target_bir_lowering=False, debug=False)
    d = {}

    def din(name, shape, dtype=F32):
        d[name] = nc.dram_tensor(name, shape, dtype, kind="ExternalInput").ap()

    din("xT", [DIM, N], F32R)
    din("maskT", [N, N])
    din("Wqkv1", [DIM, 3 * DIM], F32R)
    din("bqkv1", [3 * DIM])
    din("Wqkv2", [DIM, 3 * DIM], F32R)
    din("bqkv2", [3 * DIM])
    din("Wnn1", [DIM, DIM], F32R)
    din("bnn1", [DIM])
    din("bv1b", [128, DIM])
    din("bv2b", [128, DIM])
    din("bnnb", [128, DIM])
    din("onesb", [128, DIM], F32R)
    out_ap = nc.dram_tensor("out", [N, DIM], F32, kind="ExternalOutput").ap()

    taps = None
    if debug_taps:
        shapes = {"qt1_0": [128, N], "v1_0": [128, DIM], "o1t_0": [128, N],
                  "qt2_0": [128, N], "v2p_0": [128, H * VP],
                  "db_00": [64, 512], "o2t_0": [128, N]}
        taps = {k: nc.dram_tensor(f"tap_{k}", v, F32, kind="ExternalOutput").ap()
                for k, v in shapes.items()}

    with tile.TileContext(nc) as tc:
        if n_repeat == 1:
            with ExitStack() as ctx:
                try:
                    build_body(ctx, tc, d, out_ap, taps=taps)
                except StopIteration:
                    pass
        else:
            # Hardware loop: n_repeat executions of ONE static body, so the
            # NEFF size stays constant in n_repeat and repeat-differencing
            # isolates on-device execution (per-NEFF-size host overhead
            # cancels in T(R) - T(1)).
            with tc.For_i(0, n_repeat):
                with ExitStack() as ctx:
                    try:
                        build_body(ctx, tc, d, out_ap, taps=taps)
                    except StopIteration:
                        pass

    nc.compile()
    n = legalize_single_wait(nc)
    return nc, n


# ===========================================================================
# Host-side entry point: full inputs in, full output out.
# Sharding: pure data-parallel — B=8 batch elements, one per NeuronCore.
# ===========================================================================
import numpy as np

_CACHED = {}


def _get_program():
    if "nc" not in _CACHED:
        _CACHED["nc"] = build(n_repeat=1)[0]
    return _CACHED["nc"]


def _make_common(mask, Wqkv1, bqkv1, Wqkv2, bqkv2, Wnn1, bnn1):
    f32 = lambda a: np.ascontiguousarray(np.asarray(a, dtype=np.float32))
    bqkv1, bqkv2, bnn1 = f32(bqkv1), f32(bqkv2), f32(bnn1)
    return {
        "maskT": f32(np.asarray(mask)[0, 0].T),
        "Wqkv1": f32(Wqkv1),
        "bqkv1": bqkv1,
        "Wqkv2": f32(Wqkv2),
        "bqkv2": bqkv2,
        "Wnn1": f32(Wnn1),
        "bnn1": bnn1,
        "bv1b": f32(np.broadcast_to(bqkv1[2 * DIM :], (128, DIM))),
        "bv2b": f32(np.broadcast_to(bqkv2[2 * DIM :], (128, DIM))),
        "bnnb": f32(np.broadcast_to(bnn1, (128, DIM))),
        "onesb": np.ones((128, DIM), dtype=np.float32),
    }


def kernel(x, mask, Wqkv1, bqkv1, Wqkv2, bqkv2, Wnn1, bnn1):
    from concourse.bass_utils import run_bass_kernel_spmd

    x = np.asarray(x, dtype=np.float32)
    common = _make_common(mask, Wqkv1, bqkv1, Wqkv2, bqkv2, Wnn1, bnn1)
    in_maps = [
        {"xT": np.ascontiguousarray(x[c].T), **common} for c in range(x.shape[0])
    ]
    nc = _get_program()
    res = run_bass_kernel_spmd(nc, in_maps, core_ids=list(range(8)))
    return np.stack([res.results[c]["out"] for c in range(8)]).astype(np.float32)



# revision 3
# speedup vs baseline: 393.9256x; 48.1369x over previous
"""Bass/Tile kernel for the two-stage attention block, v3: static-instruction-
minimized for an environment where per-STATIC-instruction overhead (~40-70us)
dominates execution cost.

Key structural choices vs v2:
- All matmul operands bf16 (PE moving operand up to 1024 free) -> 416 matmuls
  (vs 736 f32r@512-free). End-to-end rel rms err ~5e-3 (gate 2e-2).
- Elementwise fused into [128, 4096] ops: one TT (mask-mult, in-place on an
  8-bank PSUM tile, stride-0 broadcast mask operand) + one ACT per
  (j-chunk, 4-head group) -> 32+32 elementwise ops per attention stage.
- 7 DMAs total (host pre-packs weights/bias/mask into per-core bundles).
- n_repeat>1 builds ONE static body inside a tc.For_i hardware loop, so the
  NEFF stays small and repeat-differencing isolates on-device time.

Layout (all SBUF tiles bf16 except biases/psum/output):
  xtb   [128, 4*1024]  x[b].T k-chunks along free
  w1    [128, 4*1536]  Wqkv1 k-chunks
  w2    [128, 4*2048]  [W2q|W2k|W2vp] k-chunks; W2vp = v-cols padded to 128
                       per head (64 data + 64 zero; bias plane supplies 1.0
                       in the zero cols -> softmax denominator rows for free)
  qk1/qk2 [128, 8*1024] Q^T chunks 0-3 (2 heads per chunk: partitions 0:64 /
                       64:128), K^T chunks 4-7
  v1    [128, 8*512]   V1 j-chunks;  v2p [128, 8*1024] padded V2 j-chunks
  p1/p2 [128, 8*8192]  P^T per j-chunk: 8 heads x 1024 tokens
  o1t/o2t [128, 4*1024] out^T c-chunks (head-pair packed: rows 0:64 head 2t,
                       64:128 head 2t+1 == feature order of the reference)
Stage-2 softmax: exp(S*SCALE - 15) with the denominator computed by the same
apply matmul via the 64 ones-columns of V2p (PSUM rows 64:128), then one
TT-divide per head.
"""

from contextlib import ExitStack

import numpy as np

import concourse.bacc as bacc
import concourse.bass as bass
import concourse.tile as tile
from concourse import mybir
from concourse.vector_clock import ScopedClock

F32 = mybir.dt.float32
BF16 = mybir.dt.bfloat16
AF = mybir.ActivationFunctionType
ALU = mybir.AluOpType

N, DIM, H, D = 1024, 512, 8, 64
SCALE = DIM**-0.5
KC = DIM // 128  # 4 contraction chunks
JC = N // 128  # 8 token chunks
EXP_BIAS = -15.0

# bias bundle column layout
BQ1, BK1, BQ2, BK2 = 0, 4, 8, 12
BV1 = 16
BV2P = BV1 + DIM
BNN = BV2P + H * 128
BEXP = BNN + DIM
BIAS_COLS = BEXP + 1


# ---------------------------------------------------------------------------
# Walrus in this container rejects instructions with >1 sync-wait.
def legalize_single_wait(nc):
    n_split = 0
    for fn in nc.m.functions:
        for blk in fn.blocks:
            insts = list(blk.instructions)
            out = []
            changed = False
            for inst in insts:
                si = inst.sync_info
                waits = list(si.on_wait) if (si is not None and si.on_wait) else []
                if len(waits) > 1:
                    changed = True
                    n_split += len(waits) - 1
                    for w in waits[:-1]:
                        nop = mybir.InstNoOp(
                            name=nc.get_next_instruction_name(),
                            sync_info=mybir.SyncInfo(on_wait=[w], on_update=[]),
                            bass_nofuse=True,
                            engine=inst.engine,
                        )
                        nc.register_instruction(nop)
                        out.append(nop)
                    si.on_wait = [waits[-1]]
                out.append(inst)
            if changed:
                blk.instructions = out
    return n_split


def _patched_drain_and_barrier(self, tick_clock, wait_clock):
    drain_inst = self.nc.sync.drain()
    wait_clock.add_sem_waits(
        drain_inst.ins, ScopedClock({None: tick_clock.global_clock})
    )
    si = drain_inst.ins.sync_info
    waits = list(si.on_wait or []) if si is not None else []
    if len(waits) > 1:
        si.on_wait = [waits[0]]
        for w in waits[1:]:
            extra = self.nc.sync.drain()
            esi = extra.ins.sync_info
            if esi is None:
                extra.ins.sync_info = mybir.SyncInfo(on_wait=[w], on_update=[])
            else:
                esi.on_wait = [w]

    self.nc.all_engine_barrier()
    assert self.sems is not None
    popped = self.nc._tile_sem_poison_stack.pop()
    assert popped is self._sem_poison
    self.nc.clear_and_free_semaphores(list(self.sems.allocated().values()))
    self.nc.all_engine_barrier()


def install_patches():
    tile.TileContext._drain_and_barrier = _patched_drain_and_barrier


# ---------------------------------------------------------------------------


def build_body(ctx: ExitStack, tc: tile.TileContext, d, out_ap, taps=None):
    nc = tc.nc

    def tap(name, ap):
        if taps is not None and name in taps:
            if ap.dtype != F32:
                ap = ap.bitcast(F32)
            nc.sync.dma_start(taps[name][:], ap)

    const_pool = ctx.enter_context(tc.tile_pool(name="const", bufs=1))
    bias = const_pool.tile([128, BIAS_COLS], F32, name="bias")
    nc.sync.dma_start(bias[:], d["biasb"][:, :])

    o1_pool = ctx.enter_context(tc.tile_pool(name="o1", bufs=1))
    o1t = o1_pool.tile([128, KC * N], BF16, name="o1t")

    # ================= stage 1 ============================================
    s1 = ExitStack()
    qkv1_pool = s1.enter_context(tc.tile_pool(name="qkv1", bufs=1))
    qk1 = qkv1_pool.tile([128, 8 * N], BF16, name="qk1")
    v1 = qkv1_pool.tile([128, JC * DIM], BF16, name="v1")
    mask_pool = s1.enter_context(tc.tile_pool(name="mask", bufs=1))
    maskb = mask_pool.tile([128, JC * N], BF16, name="maskb")
    p1_pool = s1.enter_context(tc.tile_pool(name="p1", bufs=1))
    p1 = p1_pool.tile([128, JC * H * N], BF16, name="p1")

    nc.scalar.dma_start(maskb[:], d["maskb"][:, :])

    # --- phase 1: stage-1 projections -------------------------------------
    with tc.tile_pool(name="xw1", bufs=1) as xw1_pool:
        xtb = xw1_pool.tile([128, KC * N], BF16, name="xtb")
        w1 = xw1_pool.tile([128, KC * 1536], BF16, name="w1")
        nc.sync.dma_start(xtb[:], d["xtb"][:, :])
        nc.sync.dma_start(w1[:], d["w1b"][:, :])

        ps_qk_stack = ExitStack()
        ps_qk = ps_qk_stack.enter_context(
            tc.tile_pool(name="ps_qk1", bufs=1, space="PSUM")
        )
        for t in range(8):  # 0-3: Q^T chunks, 4-7: K^T chunks
            tt = t % 4
            col0 = 0 if t < 4 else DIM
            ps = ps_qk.tile([128, N], F32, tag=f"qkps{t % 4}", name=f"qkps_{t}")
            for ic in range(2):
                for k in range(KC):
                    nc.tensor.matmul(
                        ps[:, ic * 512 : (ic + 1) * 512],
                        w1[:, k * 1536 + col0 + tt * 128 : k * 1536 + col0 + (tt + 1) * 128],
                        xtb[:, k * N + ic * 512 : k * N + (ic + 1) * 512],
                        start=(k == 0),
                        stop=(k == KC - 1),
                    )
            nc.vector.tensor_scalar_add(
                qk1[:, t * N : (t + 1) * N], ps[:], bias[:, t : t + 1]
            )
        ps_qk_stack.close()
        with tc.tile_pool(name="ps_v1", bufs=1, space="PSUM") as ps_v:
            psv = ps_v.tile([128, JC * DIM], F32, tag="vps", name="vps")
            for j in range(JC):
                for k in range(KC):
                    nc.tensor.matmul(
                        psv[:, j * DIM : (j + 1) * DIM],
                        xtb[:, k * N + j * 128 : k * N + (j + 1) * 128],
                        w1[:, k * 1536 + 2 * DIM : k * 1536 + 3 * DIM],
                        start=(k == 0),
                        stop=(k == KC - 1),
                    )
            nc.vector.tensor_tensor(
                v1[:].rearrange("p (a f) -> p a f", f=DIM),
                psv[:].rearrange("p (a f) -> p a f", f=DIM),
                bias[:, BV1 : BV1 + DIM].unsqueeze(1).broadcast_to((128, JC, DIM)),
                ALU.add,
            )
        tap("qk1", qk1[:, 0:N])
        tap("v1", v1[:, 0:DIM])

    # --- phase 2: stage-1 scores + mask + sigmoid -------------------------
    with tc.tile_pool(name="ps_s1", bufs=1, space="PSUM") as ps_s1:
        for j in range(JC):
            for half in range(2):
                ps = ps_s1.tile([128, 4 * N], F32, tag="sps", name=f"sps_{j}_{half}")
                for g in range(4):
                    h = half * 4 + g
                    t = h // 2
                    base = 64 * (h % 2)
                    for ic in range(2):
                        nc.tensor.matmul(
                            ps[:, g * N + ic * 512 : g * N + (ic + 1) * 512],
                            qk1[base : base + 64, (4 + t) * N + j * 128 : (4 + t) * N + (j + 1) * 128],
                            qk1[base : base + 64, t * N + ic * 512 : t * N + (ic + 1) * 512],
                            start=True,
                            stop=True,
                        )
                ps3 = ps[:].rearrange("p (a f) -> p a f", a=4)
                nc.vector.tensor_tensor(
                    ps3,
                    ps3,
                    maskb[:, j * N : (j + 1) * N].unsqueeze(1).broadcast_to((128, 4, N)),
                    ALU.mult,
                )
                nc.scalar.activation(
                    p1[:, j * H * N + half * 4 * N : j * H * N + (half + 1) * 4 * N],
                    ps[:],
                    AF.Sigmoid,
                )
        tap("p1", p1[:, 0:N].bitcast(BF16))

    # --- phase 3: stage-1 apply ------------------------------------------
    with tc.tile_pool(name="ps_a1", bufs=1, space="PSUM") as ps_a1:
        ps = ps_a1.tile([128, 4 * N], F32, tag="aps", name="aps")
        for pair in range(4):
            for j in range(JC):
                for h in (2 * pair, 2 * pair + 1):
                    base = 64 * (h % 2)
                    for ic in range(2):
                        nc.tensor.matmul(
                            ps[base : base + 64, pair * N + ic * 512 : pair * N + (ic + 1) * 512],
                            v1[:, j * DIM + h * D : j * DIM + (h + 1) * D],
                            p1[:, j * H * N + h * N + ic * 512 : j * H * N + h * N + (ic + 1) * 512],
                            start=(j == 0),
                            stop=(j == JC - 1),
                        )
        nc.scalar.copy(o1t[:], ps[:])
    s1.close()
    tap("o1t", o1t[:, 0:N])

    # ================= stage 2 ============================================
    o2_pool = ctx.enter_context(tc.tile_pool(name="o2", bufs=1))
    o2t = o2_pool.tile([128, KC * N], BF16, name="o2t")
    s2 = ExitStack()
    qkv2_pool = s2.enter_context(tc.tile_pool(name="qkv2", bufs=1))
    qk2 = qkv2_pool.tile([128, 8 * N], BF16, name="qk2")
    v2p = qkv2_pool.tile([128, JC * H * 128], BF16, name="v2p")
    p2_pool = s2.enter_context(tc.tile_pool(name="p2", bufs=1))
    p2 = p2_pool.tile([128, JC * H * N], BF16, name="p2")

    # --- phase 4: stage-2 projections -------------------------------------
    with tc.tile_pool(name="w2p", bufs=1) as w2_pool:
        w2 = w2_pool.tile([128, KC * 2048], BF16, name="w2")
        nc.sync.dma_start(w2[:], d["w2b"][:, :])
        ps_qk2_stack = ExitStack()
        ps_qk2 = ps_qk2_stack.enter_context(
            tc.tile_pool(name="ps_qk2", bufs=1, space="PSUM")
        )
        for t in range(8):
            tt = t % 4
            col0 = 0 if t < 4 else DIM
            bcol = (BQ2 + tt) if t < 4 else (BK2 + tt)
            ps = ps_qk2.tile([128, N], F32, tag=f"qk2ps{t % 4}", name=f"qk2ps_{t}")
            for ic in range(2):
                for k in range(KC):
                    nc.tensor.matmul(
                        ps[:, ic * 512 : (ic + 1) * 512],
                        w2[:, k * 2048 + col0 + tt * 128 : k * 2048 + col0 + (tt + 1) * 128],
                        o1t[:, k * N + ic * 512 : k * N + (ic + 1) * 512],
                        start=(k == 0),
                        stop=(k == KC - 1),
                    )
            nc.vector.tensor_scalar_add(
                qk2[:, t * N : (t + 1) * N], ps[:], bias[:, bcol : bcol + 1]
            )
        ps_qk2_stack.close()
        with tc.tile_pool(name="ps_v2", bufs=1, space="PSUM") as ps_v2:
          for w in range(2):
            ps = ps_v2.tile([128, 4096], F32, tag="v2ps", name=f"v2ps_{w}")
            for jj in range(4):
                j = w * 4 + jj
                for vh in range(2):
                    for k in range(KC):
                        nc.tensor.matmul(
                            ps[:, jj * 1024 + vh * 512 : jj * 1024 + (vh + 1) * 512],
                            o1t[:, k * N + j * 128 : k * N + (j + 1) * 128],
                            w2[:, k * 2048 + 1024 + vh * 512 : k * 2048 + 1024 + (vh + 1) * 512],
                            start=(k == 0),
                            stop=(k == KC - 1),
                        )
            nc.vector.tensor_tensor(
                v2p[:, w * 4096 : (w + 1) * 4096].rearrange("p (a f) -> p a f", a=4),
                ps[:].rearrange("p (a f) -> p a f", a=4),
                bias[:, BV2P : BV2P + 1024].unsqueeze(1).broadcast_to((128, 4, 1024)),
                ALU.add,
            )
        tap("qk2", qk2[:, 0:N])
        tap("v2p", v2p[:, 0:1024])

    # --- phase 5: stage-2 scores + exp ------------------------------------
    with tc.tile_pool(name="ps_s2", bufs=1, space="PSUM") as ps_s2:
        for j in range(JC):
            for half in range(2):
                ps = ps_s2.tile([128, 4 * N], F32, tag="s2ps", name=f"s2ps_{j}_{half}")
                for g in range(4):
                    h = half * 4 + g
                    t = h // 2
                    base = 64 * (h % 2)
                    for ic in range(2):
                        nc.tensor.matmul(
                            ps[:, g * N + ic * 512 : g * N + (ic + 1) * 512],
                            qk2[base : base + 64, (4 + t) * N + j * 128 : (4 + t) * N + (j + 1) * 128],
                            qk2[base : base + 64, t * N + ic * 512 : t * N + (ic + 1) * 512],
                            start=True,
                            stop=True,
                        )
                nc.scalar.activation(
                    p2[:, j * H * N + half * 4 * N : j * H * N + (half + 1) * 4 * N],
                    ps[:],
                    AF.Exp,
                    bias=bias[:, BEXP : BEXP + 1],
                    scale=SCALE,
                )
        tap("p2", p2[:, 0:N].bitcast(BF16))

    # --- phase 6: stage-2 apply + normalize -------------------------------
    with tc.tile_pool(name="ps_a2", bufs=1, space="PSUM") as ps_a2, \
         tc.tile_pool(name="dent", bufs=1) as den_pool:
        for w in range(2):
            ps = ps_a2.tile([128, 4 * N], F32, tag="a2ps", name=f"a2ps_{w}")
            for g in range(4):
                h = 2 * g + w
                for ic in range(2):
                    for j in range(JC):
                        nc.tensor.matmul(
                            ps[:, g * N + ic * 512 : g * N + (ic + 1) * 512],
                            v2p[:, j * 1024 + h * 128 : j * 1024 + (h + 1) * 128],
                            p2[:, j * H * N + h * N + ic * 512 : j * H * N + h * N + (ic + 1) * 512],
                            start=(j == 0),
                            stop=(j == JC - 1),
                        )
            den = den_pool.tile([64, 4 * N], F32, tag="den", name=f"den_{w}")
            nc.vector.reciprocal(den[:], ps[64:128, :])
            nc.vector.tensor_tensor(
                o2t[64 * w : 64 * w + 64, :], ps[0:64, :], den[:], ALU.mult
            )
    s2.close()
    tap("o2t", o2t[:, 0:N])

    # --- phase 7: output projection ---------------------------------------
    with tc.tile_pool(name="wnn", bufs=1) as wnn_pool, \
         tc.tile_pool(name="ps_o", bufs=1, space="PSUM") as ps_o:
        wnn = wnn_pool.tile([128, KC * DIM], BF16, name="wnn")
        ob = wnn_pool.tile([128, JC * DIM], F32, name="ob")
        nc.sync.dma_start(wnn[:], d["wnnb"][:, :])
        pso = ps_o.tile([128, JC * DIM], F32, tag="ops", name="ops")
        for i8 in range(JC):
            for k in range(KC):
                nc.tensor.matmul(
                    pso[:, i8 * DIM : (i8 + 1) * DIM],
                    o2t[:, k * N + i8 * 128 : k * N + (i8 + 1) * 128],
                    wnn[:, k * DIM : (k + 1) * DIM],
                    start=(k == 0),
                    stop=(k == KC - 1),
                )
        nc.vector.tensor_tensor(
            ob[:].rearrange("p (a f) -> p a f", f=DIM),
            pso[:].rearrange("p (a f) -> p a f", f=DIM),
            bias[:, BNN : BNN + DIM].unsqueeze(1).broadcast_to((128, JC, DIM)),
            ALU.add,
        )
        nc.sync.dma_start(
            out_ap.rearrange("(a p) c -> p a c", p=128),
            ob[:].rearrange("p (a c) -> p a c", a=JC),
        )


def build(n_repeat: int = 1, debug_taps: bool = False):
    install_patches()
    nc = bacc.Bacc("TRN2", target_bir_lowering=False, debug=False)
    d = {}

    def din(name, shape, dtype):
        d[name] = nc.dram_tensor(name, shape, dtype, kind="ExternalInput").ap()

    din("xtb", [128, KC * N], BF16)
    din("w1b", [128, KC * 1536], BF16)
    din("w2b", [128, KC * 2048], BF16)
    din("wnnb", [128, KC * DIM], BF16)
    din("maskb", [128, JC * N], BF16)
    din("biasb", [128, BIAS_COLS], F32)
    out_ap = nc.dram_tensor("out", [N, DIM], F32, kind="ExternalOutput").ap()

    taps = None
    if debug_taps:
        shapes = {
            "qk1": [128, N], "v1": [128, DIM], "p1": [128, N // 2],
            "o1t": [128, N], "qk2": [128, N], "v2p": [128, 1024],
            "p2": [128, N // 2], "o2t": [128, N],
        }
        taps = {k: nc.dram_tensor(f"tap_{k}", v, F32, kind="ExternalOutput").ap()
                for k, v in shapes.items()}

    with tile.TileContext(nc) as tc:
        if n_repeat == 1:
            with ExitStack() as ctx:
                build_body(ctx, tc, d, out_ap, taps=taps)
        else:
            with tc.For_i(0, n_repeat):
                with ExitStack() as ctx:
                    build_body(ctx, tc, d, out_ap, taps=taps)

    nc.compile()
    n = legalize_single_wait(nc)
    return nc, n


# ===========================================================================
# Host-side entry point: full inputs in, full output out.
# Sharding: pure data-parallel — B=8 batch elements, one per NeuronCore.
# ===========================================================================

_CACHED = {}


def _get_program():
    if "nc" not in _CACHED:
        _CACHED["nc"] = build(n_repeat=1)[0]
    return _CACHED["nc"]


def _make_common(mask, Wqkv1, bqkv1, Wqkv2, bqkv2, Wnn1, bnn1):
    import ml_dtypes

    bf16 = ml_dtypes.bfloat16
    f32 = lambda a: np.asarray(a, dtype=np.float32)
    Wqkv1, Wqkv2, Wnn1 = f32(Wqkv1), f32(Wqkv2), f32(Wnn1)
    bqkv1, bqkv2, bnn1 = f32(bqkv1), f32(bqkv2), f32(bnn1)
    maskT = f32(np.asarray(mask)[0, 0]).T  # [j, i]

    def kchunks(w):  # [512, C] -> [128, 4*C] bf16
        return np.ascontiguousarray(
            np.concatenate([w[k * 128 : (k + 1) * 128, :] for k in range(KC)], axis=1)
        ).astype(bf16)

    W2q, W2k, W2v = Wqkv2[:, :DIM], Wqkv2[:, DIM : 2 * DIM], Wqkv2[:, 2 * DIM :]
    W2vp = np.zeros((DIM, H * 128), np.float32)
    for h in range(H):
        W2vp[:, h * 128 : h * 128 + D] = W2v[:, h * D : (h + 1) * D]
    w2full = np.concatenate([W2q, W2k, W2vp], axis=1)  # [512, 2048]

    maskc = np.ascontiguousarray(
        np.concatenate([maskT[j * 128 : (j + 1) * 128, :] for j in range(JC)], axis=1)
    ).astype(bf16)

    biasb = np.zeros((128, BIAS_COLS), np.float32)
    for t in range(4):
        biasb[:, BQ1 + t] = bqkv1[t * 128 : (t + 1) * 128]
        biasb[:, BK1 + t] = bqkv1[DIM + t * 128 : DIM + (t + 1) * 128]
        biasb[:, BQ2 + t] = bqkv2[t * 128 : (t + 1) * 128]
        biasb[:, BK2 + t] = bqkv2[DIM + t * 128 : DIM + (t + 1) * 128]
    biasb[:, BV1 : BV1 + DIM] = bqkv1[2 * DIM :][None, :]
    bv2p = np.zeros(H * 128, np.float32)
    for h in range(H):
        bv2p[h * 128 : h * 128 + D] = bqkv2[2 * DIM + h * D : 2 * DIM + (h + 1) * D]
        bv2p[h * 128 + D : (h + 1) * 128] = 1.0
    biasb[:, BV2P : BV2P + H * 128] = bv2p[None, :]
    biasb[:, BNN : BNN + DIM] = bnn1[None, :]
    biasb[:, BEXP] = EXP_BIAS

    return {
        "w1b": kchunks(Wqkv1),
        "w2b": kchunks(w2full),
        "wnnb": kchunks(Wnn1),
        "maskb": maskc,
        "biasb": biasb,
    }


def _make_xtb(xb):
    import ml_dtypes

    bf16 = ml_dtypes.bfloat16
    xT = np.asarray(xb, dtype=np.float32).T  # [512, 1024]
    return np.ascontiguousarray(
        np.concatenate([xT[k * 128 : (k + 1) * 128, :] for k in range(KC)], axis=1)
    ).astype(bf16)


def kernel(x, mask, Wqkv1, bqkv1, Wqkv2, bqkv2, Wnn1, bnn1):
    from concourse.bass_utils import run_bass_kernel_spmd

    x = np.asarray(x, dtype=np.float32)
    common = _make_common(mask, Wqkv1, bqkv1, Wqkv2, bqkv2, Wnn1, bnn1)
    in_maps = [{"xtb": _make_xtb(x[c]), **common} for c in range(x.shape[0])]
    nc = _get_program()
    res = run_bass_kernel_spmd(nc, in_maps, core_ids=list(range(8)))
    return np.stack([res.results[c]["out"] for c in range(8)]).astype(np.float32)
